# revision 7
# baseline (speedup 1.0000x reference)
"""DistMult edge-scoring kernel for Trainium2 (8 NeuronCores, SPMD).

score[j] = sum_d emb_A[a_idx[j], d] * k[d] * emb_B[b_idx[j], d]
for 9E pairs: E positive edges, 4E head-corrupted, 4E tail-corrupted.

Strategy (data-parallel over edge pairs, tables replicated):
- Host prescales emb_Bk = emb_B * k, so each pair's score is a plain
  dot product of one emb_A row and one emb_Bk row.
- Rows are fetched with gpsimd.dma_gather (num_idxs rows per call, int16
  chunk-local indices). Tables are split into 4 chunks of 25000 rows;
  pairs are sorted by (a_chunk, b_chunk) into 16 groups on the host.
- The 900K pairs are dealt round-robin across the 8 cores in 128-pair
  sub-slots, so every core runs an identical instruction stream (true
  SPMD). The program is built after seeing the data; compile is cached
  on the 16-group slot signature.
- Compute: one fused scalar_tensor_tensor (mul + accumulate-reduce) per
  128-pair slot on the vector engine.
- Host inverse-permutes the scores back to reference order.
"""

import numpy as np

# problem constants
N_A = 100000
N_B = 100000
D = 128
E = 100000
NEG = 4
NCORES = 8

P = 128
CHUNK = 25000          # table rows per int16-indexable chunk
NCHUNKS = 4
BATCH_SLOTS = 8        # max 128-pair slots per dma_gather batch
                       # (num_idxs=1024 verified on HW; 2048+ crashes the runtime)
SUB = P * NCORES       # pairs per dealt slot-row (1024)

_CACHED = {}


def _build_program(batches, total_slots):
    """batches: list of (a_chunk, b_chunk, n_slots). Same for every core."""
    import concourse.tile as tile
    from concourse import bacc, mybir

    f32 = mybir.dt.float32
    i16 = mybir.dt.int16
    mult = mybir.AluOpType.mult

    total_idx_cols = sum(n * 8 for _, _, n in batches)  # n*128/16 per batch

    nc = bacc.Bacc("TRN2", target_bir_lowering=False, debug=False,
                   num_devices=NCORES)
    embA = nc.dram_tensor("emb_a", [N_A, D], f32, kind="ExternalInput").ap()
    embBk = nc.dram_tensor("emb_bk", [N_B, D], f32, kind="ExternalInput").ap()
    idxA_d = nc.dram_tensor("idx_a", [P, total_idx_cols], i16,
                            kind="ExternalInput").ap()
    idxB_d = nc.dram_tensor("idx_b", [P, total_idx_cols], i16,
                            kind="ExternalInput").ap()
    s_out = nc.dram_tensor("scores", [P, total_slots], f32,
                           kind="ExternalOutput").ap()

    with tile.TileContext(nc) as tc:
        with (
            tc.tile_pool(name="idx", bufs=2) as idx_pool,
            tc.tile_pool(name="gather", bufs=2) as g_pool,
            tc.tile_pool(name="trash", bufs=2) as trash_pool,
            tc.tile_pool(name="scores", bufs=1) as s_pool,
        ):
            scores = s_pool.tile([P, total_slots], f32)
            col = 0
            slot = 0
            for (ca, cb, n) in batches:
                nidx = n * P
                cols = n * 8
                idxA_sb = idx_pool.tile([P, BATCH_SLOTS * 8], i16, tag="ia")
                nc.sync.dma_start(idxA_sb[:, 0:cols],
                                  idxA_d[:, col:col + cols])
                idxB_sb = idx_pool.tile([P, BATCH_SLOTS * 8], i16, tag="ib")
                nc.sync.dma_start(idxB_sb[:, 0:cols],
                                  idxB_d[:, col:col + cols])

                A = g_pool.tile([P, BATCH_SLOTS * D], f32, tag="A")
                nc.gpsimd.dma_gather(
                    out_ap=A[:, 0:n * D].rearrange("p (g d) -> p g d", d=D),
                    in_ap=embA[ca * CHUNK:min((ca + 1) * CHUNK, N_A), :],
                    idxs_ap=idxA_sb[:, 0:cols],
                    num_idxs=nidx, num_idxs_reg=nidx, elem_size=D)
                B = g_pool.tile([P, BATCH_SLOTS * D], f32, tag="B")
                nc.gpsimd.dma_gather(
                    out_ap=B[:, 0:n * D].rearrange("p (g d) -> p g d", d=D),
                    in_ap=embBk[cb * CHUNK:min((cb + 1) * CHUNK, N_B), :],
                    idxs_ap=idxB_sb[:, 0:cols],
                    num_idxs=nidx, num_idxs_reg=nidx, elem_size=D)

                for s in range(n):
                    tr = trash_pool.tile([P, D], f32, tag="tr")
                    nc.vector.scalar_tensor_tensor(
                        out=tr[:],
                        in0=A[:, s * D:(s + 1) * D], scalar=1.0,
                        in1=B[:, s * D:(s + 1) * D],
                        op0=mult, op1=mult,
                        accum_out=scores[:, slot + s:slot + s + 1])
                col += cols
                slot += n
            nc.sync.dma_start(s_out[:], scores[:])

    nc.compile()
    return nc


def _wrap_idx(flat_idx):
    """[N] int16 (N % 128 == 0) -> [128, N/16] wrapped + Q7-core-replicated."""
    n = flat_idx.shape[0]
    w16 = flat_idx.reshape(n // 16, 16).T          # [16, N/16]
    return np.tile(w16, (8, 1))                    # [128, N/16]


def kernel(emb_A, emb_B, rel_kernel, edge_pos, head_batch, tail_batch):
    from concourse.bass_utils import run_bass_kernel_spmd

    emb_A = np.ascontiguousarray(np.asarray(emb_A, dtype=np.float32))
    emb_Bk = np.ascontiguousarray(
        np.asarray(emb_B, dtype=np.float32)
        * np.asarray(rel_kernel, dtype=np.float32)[0][None, :])
    ep = np.asarray(edge_pos, dtype=np.int64)
    hb = np.asarray(head_batch, dtype=np.int64)
    tb = np.asarray(tail_batch, dtype=np.int64)

    # global pair list in reference output order
    a_all = np.concatenate([ep[0], hb.reshape(-1), np.repeat(ep[0], NEG)])
    b_all = np.concatenate([ep[1], np.repeat(ep[1], NEG), tb.reshape(-1)])
    npairs = a_all.shape[0]                       # 9E
    key = (a_all // CHUNK) * NCHUNKS + (b_all // CHUNK)
    order = np.argsort(key, kind="stable")
    a_s, b_s, pos_s = a_all[order], b_all[order], order
    counts = np.bincount(key, minlength=NCHUNKS * NCHUNKS)

    # per-group padded layout; identical structure for all cores
    group_slots = [int(-(-c // SUB)) for c in counts]     # ceil(c/1024)
    batches = []          # (a_chunk, b_chunk, n_slots) per gather batch
    for g, gs in enumerate(group_slots):
        left = gs
        while left > 0:
            n = min(left, BATCH_SLOTS)
            batches.append((g // NCHUNKS, g % NCHUNKS, n))
            left -= n
    total_slots = sum(group_slots)

    # build per-core chunk-local idx arrays and the outpos map
    # group g's padded pairs: sub-slot i (128 pairs) -> core i%8, slot i//8
    idx_a_cores = [[] for _ in range(NCORES)]
    idx_b_cores = [[] for _ in range(NCORES)]
    outpos_cores = [[] for _ in range(NCORES)]
    start = 0
    for g, gs in enumerate(group_slots):
        cnt = int(counts[g])
        padded = gs * SUB
        ga = np.zeros(padded, dtype=np.int16)
        gb = np.zeros(padded, dtype=np.int16)
        gp = np.full(padded, -1, dtype=np.int64)
        ga[:cnt] = (a_s[start:start + cnt] - (g // NCHUNKS) * CHUNK).astype(np.int16)
        gb[:cnt] = (b_s[start:start + cnt] - (g % NCHUNKS) * CHUNK).astype(np.int16)
        gp[:cnt] = pos_s[start:start + cnt]
        start += cnt
        # [slots, cores, 128]
        ga = ga.reshape(gs, NCORES, P)
        gb = gb.reshape(gs, NCORES, P)
        gp = gp.reshape(gs, NCORES, P)
        for c in range(NCORES):
            idx_a_cores[c].append(ga[:, c, :].reshape(-1))
            idx_b_cores[c].append(gb[:, c, :].reshape(-1))
            outpos_cores[c].append(gp[:, c, :].reshape(-1))

    in_maps = []
    for c in range(NCORES):
        fa = np.concatenate(idx_a_cores[c])
        fb = np.concatenate(idx_b_cores[c])
        in_maps.append({
            "emb_a": emb_A,
            "emb_bk": emb_Bk,
            "idx_a": np.ascontiguousarray(_wrap_idx(fa)),
            "idx_b": np.ascontiguousarray(_wrap_idx(fb)),
        })

    sig = tuple(group_slots)
    if _CACHED.get("sig") != sig:
        _CACHED["nc"] = _build_program(batches, total_slots)
        _CACHED["sig"] = sig
    nc = _CACHED["nc"]
    _CACHED["batches"] = batches
    _CACHED["total_slots"] = total_slots

    _CACHED["in_maps"] = in_maps
    res = run_bass_kernel_spmd(nc, in_maps, core_ids=list(range(NCORES)))
    _CACHED["last_results"] = res

    out = np.empty(npairs, dtype=np.float32)
    for c in range(NCORES):
        flat = res.results[c]["scores"].T.reshape(-1)   # j = slot*128 + p
        op = np.concatenate(outpos_cores[c])
        valid = op >= 0
        out[op[valid]] = flat[valid]
    return out


# revision 11
# speedup vs baseline: 3.4667x; 3.4667x over previous
"""DistMult edge-scoring kernel for Trainium2 (8 NeuronCores, SPMD).

score[j] = sum_d emb_A[a_idx[j], d] * k[d] * emb_B[b_idx[j], d]
for 9E pairs: E positive edges, 4E head-corrupted, 4E tail-corrupted.

Strategy (data-parallel over edge pairs, tables replicated):
- Host prescales emb_Bk = emb_B * k, so each pair's score is a plain
  dot product of one emb_A row and one emb_Bk row.
- Rows are fetched with gpsimd.dma_gather (num_idxs rows per call, int16
  chunk-local indices). Tables are split into 4 chunks of 25000 rows;
  pairs are sorted by (a_chunk, b_chunk) into 16 groups on the host.
- The 900K pairs are dealt round-robin across the 8 cores in 128-pair
  sub-slots, so every core runs an identical instruction stream (true
  SPMD). The program is built after seeing the data; compile is cached
  on the 16-group slot signature.
- Compute: one fused scalar_tensor_tensor (mul + accumulate-reduce) per
  128-pair slot on the vector engine.
- Host inverse-permutes the scores back to reference order.
"""

import numpy as np

# problem constants
N_A = 100000
N_B = 100000
D = 128
E = 100000
NEG = 4
NCORES = 8

P = 128
CHUNK = 25000          # table rows per int16-indexable chunk
NCHUNKS = 4
BATCH_SLOTS = 8        # max 128-pair slots per dma_gather batch
                       # (num_idxs=1024 verified on HW; 2048+ crashes the runtime)
SUB = P * NCORES       # pairs per dealt slot-row (1024)

_CACHED = {}


def _build_program(batches, total_slots):
    """batches: list of (a_chunk, b_chunk, n_slots). Same for every core."""
    import concourse.tile as tile
    from concourse import bacc, mybir

    f32 = mybir.dt.float32
    i16 = mybir.dt.int16
    mult = mybir.AluOpType.mult

    total_idx_cols = sum(n * 8 for _, _, n in batches)  # n*128/16 per batch

    nc = bacc.Bacc("TRN2", target_bir_lowering=False, debug=False,
                   num_devices=NCORES, num_swdge_queues=4)
    embA = nc.dram_tensor("emb_a", [N_A, D], f32, kind="ExternalInput").ap()
    embBk = nc.dram_tensor("emb_bk", [N_B, D], f32, kind="ExternalInput").ap()
    idxA_d = nc.dram_tensor("idx_a", [P, total_idx_cols], i16,
                            kind="ExternalInput").ap()
    idxB_d = nc.dram_tensor("idx_b", [P, total_idx_cols], i16,
                            kind="ExternalInput").ap()
    s_out = nc.dram_tensor("scores", [P, total_slots], f32,
                           kind="ExternalOutput").ap()

    with tile.TileContext(nc) as tc:
        with (
            tc.tile_pool(name="idx", bufs=4) as idx_pool,
            tc.tile_pool(name="gather", bufs=4) as g_pool,
            tc.tile_pool(name="trash", bufs=2) as trash_pool,
            tc.tile_pool(name="scores", bufs=1) as s_pool,
        ):
            scores = s_pool.tile([P, total_slots], f32)
            col = 0
            slot = 0
            for bi, (ca, cb, n) in enumerate(batches):
                q = bi % 4
                nidx = n * P
                cols = n * 8
                idxA_sb = idx_pool.tile([P, BATCH_SLOTS * 8], i16, tag="ia")
                nc.sync.dma_start(idxA_sb[:, 0:cols],
                                  idxA_d[:, col:col + cols])
                idxB_sb = idx_pool.tile([P, BATCH_SLOTS * 8], i16, tag="ib")
                nc.sync.dma_start(idxB_sb[:, 0:cols],
                                  idxB_d[:, col:col + cols])

                A = g_pool.tile([P, BATCH_SLOTS * D], f32, tag="A")
                nc.gpsimd.dma_gather(
                    out_ap=A[:, 0:n * D].rearrange("p (g d) -> p g d", d=D),
                    in_ap=embA[ca * CHUNK:min((ca + 1) * CHUNK, N_A), :],
                    idxs_ap=idxA_sb[:, 0:cols],
                    num_idxs=nidx, num_idxs_reg=nidx, elem_size=D,
                    queue_num=q)
                B = g_pool.tile([P, BATCH_SLOTS * D], f32, tag="B")
                nc.gpsimd.dma_gather(
                    out_ap=B[:, 0:n * D].rearrange("p (g d) -> p g d", d=D),
                    in_ap=embBk[cb * CHUNK:min((cb + 1) * CHUNK, N_B), :],
                    idxs_ap=idxB_sb[:, 0:cols],
                    num_idxs=nidx, num_idxs_reg=nidx, elem_size=D,
                    queue_num=q)

                for s in range(n):
                    tr = trash_pool.tile([P, D], f32, tag="tr")
                    nc.vector.scalar_tensor_tensor(
                        out=tr[:],
                        in0=A[:, s * D:(s + 1) * D], scalar=1.0,
                        in1=B[:, s * D:(s + 1) * D],
                        op0=mult, op1=mult,
                        accum_out=scores[:, slot + s:slot + s + 1])
                col += cols
                slot += n
            nc.sync.dma_start(s_out[:], scores[:])

    nc.compile()
    return nc


def _wrap_idx(flat_idx):
    """[N] int16 (N % 128 == 0) -> [128, N/16] wrapped + Q7-core-replicated."""
    n = flat_idx.shape[0]
    w16 = flat_idx.reshape(n // 16, 16).T          # [16, N/16]
    return np.tile(w16, (8, 1))                    # [128, N/16]


def kernel(emb_A, emb_B, rel_kernel, edge_pos, head_batch, tail_batch):
    from concourse.bass_utils import run_bass_kernel_spmd

    emb_A = np.ascontiguousarray(np.asarray(emb_A, dtype=np.float32))
    emb_Bk = np.ascontiguousarray(
        np.asarray(emb_B, dtype=np.float32)
        * np.asarray(rel_kernel, dtype=np.float32)[0][None, :])
    ep = np.asarray(edge_pos, dtype=np.int64)
    hb = np.asarray(head_batch, dtype=np.int64)
    tb = np.asarray(tail_batch, dtype=np.int64)

    # global pair list in reference output order
    a_all = np.concatenate([ep[0], hb.reshape(-1), np.repeat(ep[0], NEG)])
    b_all = np.concatenate([ep[1], np.repeat(ep[1], NEG), tb.reshape(-1)])
    npairs = a_all.shape[0]                       # 9E
    key = (a_all // CHUNK) * NCHUNKS + (b_all // CHUNK)
    order = np.argsort(key, kind="stable")
    a_s, b_s, pos_s = a_all[order], b_all[order], order
    counts = np.bincount(key, minlength=NCHUNKS * NCHUNKS)

    # per-group padded layout; identical structure for all cores
    group_slots = [int(-(-c // SUB)) for c in counts]     # ceil(c/1024)
    batches = []          # (a_chunk, b_chunk, n_slots) per gather batch
    for g, gs in enumerate(group_slots):
        left = gs
        while left > 0:
            n = min(left, BATCH_SLOTS)
            batches.append((g // NCHUNKS, g % NCHUNKS, n))
            left -= n
    total_slots = sum(group_slots)

    # build per-core chunk-local idx arrays and the outpos map
    # group g's padded pairs: sub-slot i (128 pairs) -> core i%8, slot i//8
    idx_a_cores = [[] for _ in range(NCORES)]
    idx_b_cores = [[] for _ in range(NCORES)]
    outpos_cores = [[] for _ in range(NCORES)]
    start = 0
    for g, gs in enumerate(group_slots):
        cnt = int(counts[g])
        padded = gs * SUB
        ga = np.zeros(padded, dtype=np.int16)
        gb = np.zeros(padded, dtype=np.int16)
        gp = np.full(padded, -1, dtype=np.int64)
        ga[:cnt] = (a_s[start:start + cnt] - (g // NCHUNKS) * CHUNK).astype(np.int16)
        gb[:cnt] = (b_s[start:start + cnt] - (g % NCHUNKS) * CHUNK).astype(np.int16)
        gp[:cnt] = pos_s[start:start + cnt]
        start += cnt
        # [slots, cores, 128]
        ga = ga.reshape(gs, NCORES, P)
        gb = gb.reshape(gs, NCORES, P)
        gp = gp.reshape(gs, NCORES, P)
        for c in range(NCORES):
            idx_a_cores[c].append(ga[:, c, :].reshape(-1))
            idx_b_cores[c].append(gb[:, c, :].reshape(-1))
            outpos_cores[c].append(gp[:, c, :].reshape(-1))

    in_maps = []
    for c in range(NCORES):
        fa = np.concatenate(idx_a_cores[c])
        fb = np.concatenate(idx_b_cores[c])
        in_maps.append({
            "emb_a": emb_A,
            "emb_bk": emb_Bk,
            "idx_a": np.ascontiguousarray(_wrap_idx(fa)),
            "idx_b": np.ascontiguousarray(_wrap_idx(fb)),
        })

    sig = tuple(group_slots)
    if _CACHED.get("sig") != sig:
        _CACHED["nc"] = _build_program(batches, total_slots)
        _CACHED["sig"] = sig
    nc = _CACHED["nc"]
    _CACHED["batches"] = batches
    _CACHED["total_slots"] = total_slots

    _CACHED["in_maps"] = in_maps
    res = run_bass_kernel_spmd(nc, in_maps, core_ids=list(range(NCORES)))
    _CACHED["last_results"] = res

    out = np.empty(npairs, dtype=np.float32)
    for c in range(NCORES):
        flat = res.results[c]["scores"].T.reshape(-1)   # j = slot*128 + p
        op = np.concatenate(outpos_cores[c])
        valid = op >= 0
        out[op[valid]] = flat[valid]
    return out


# revision 14
# speedup vs baseline: 5.0281x; 1.4504x over previous
"""DistMult edge-scoring kernel for Trainium2 (8 NeuronCores, SPMD).

score[j] = sum_d emb_A[a_idx[j], d] * k[d] * emb_B[b_idx[j], d]
for 9E pairs: E positive edges, 4E head-corrupted, 4E tail-corrupted.

Strategy (v3, hybrid dense/gather — exploits the repeat structure):
- The positive-edge rows and the repeated rows (b-side of head mode,
  a-side of tail mode, both k-prescaled on the host) are uploaded as
  DENSE per-pair arrays and streamed with plain HWDGE DMA.
- Only the corrupt heads/tails are gathered on-device via
  gpsimd.dma_gather (int16 chunk-local indices, tables split in 4
  chunks of 25000 rows, pairs sorted by chunk on the host). Gathers
  round-robin over 4 SWDGE queues (descriptor generation on the Q7
  cores is the bottleneck; 4 queues parallelize it).
- All 9E pairs are dealt round-robin across the 8 cores in 128-pair
  sub-slots so every core runs an identical instruction stream (true
  SPMD). The program is built after seeing the data; compile is cached
  on the group-slot signature.
- Compute: one fused scalar_tensor_tensor (mul + accumulate-reduce) per
  128-pair slot on the vector engine.
- Host inverse-permutes the scores back to reference order.
"""

import numpy as np

# problem constants
N_A = 100000
N_B = 100000
D = 128
E = 100000
NEG = 4
NCORES = 8

P = 128
CHUNK = 25000          # table rows per int16-indexable chunk
NCHUNKS = 4
BATCH_SLOTS = 8        # 128-pair slots per batch (num_idxs=1024 HW ceiling)
SUB = P * NCORES       # pairs per dealt slot-row (1024)

_CACHED = {}


def _build_program(pos_slots, head_slots, tail_slots):
    """head_slots/tail_slots: per-chunk slot counts (len 4). Same for all
    cores. Program: pos (dense+dense), head (gather-a + dense-b), tail
    (dense-a + gather-b)."""
    import concourse.tile as tile
    from concourse import bacc, mybir

    f32 = mybir.dt.float32
    i16 = mybir.dt.int16
    mult = mybir.AluOpType.mult

    nh = sum(head_slots)
    nt = sum(tail_slots)
    total_slots = pos_slots + nh + nt

    nc = bacc.Bacc("TRN2", target_bir_lowering=False, debug=False,
                   num_devices=NCORES, num_swdge_queues=4)
    embA = nc.dram_tensor("emb_a", [N_A, D], f32, kind="ExternalInput").ap()
    embB = nc.dram_tensor("emb_b", [N_B, D], f32, kind="ExternalInput").ap()
    pos_a_d = nc.dram_tensor("pos_a", [P, pos_slots * D], f32,
                             kind="ExternalInput").ap()
    pos_b_d = nc.dram_tensor("pos_b", [P, pos_slots * D], f32,
                             kind="ExternalInput").ap()
    hidx_d = nc.dram_tensor("head_idx", [P, nh * 8], i16,
                            kind="ExternalInput").ap()
    hdense_d = nc.dram_tensor("head_dense", [P, nh * D], f32,
                              kind="ExternalInput").ap()
    tidx_d = nc.dram_tensor("tail_idx", [P, nt * 8], i16,
                            kind="ExternalInput").ap()
    tdense_d = nc.dram_tensor("tail_dense", [P, nt * D], f32,
                              kind="ExternalInput").ap()
    s_out = nc.dram_tensor("scores", [P, total_slots], f32,
                           kind="ExternalOutput").ap()

    # (table_ap, chunk, idx dram, dense dram, idx col0, dense col0, n_slots)
    gather_batches = []

    def section_batches(slots_per_chunk, idx_d, dense_d, table):
        out = []
        col = 0
        for c, gs in enumerate(slots_per_chunk):
            left = gs
            while left > 0:
                n = min(left, BATCH_SLOTS)
                out.append((table, c, idx_d, dense_d, col, n))
                col += n
                left -= n
        return out

    hb = section_batches(head_slots, hidx_d, hdense_d, embA)
    tb = section_batches(tail_slots, tidx_d, tdense_d, embB)
    # interleave head/tail so both tables' gathers spread over queues
    gather_batches = [b for pair in
                      zip(hb + [None] * len(tb), tb + [None] * len(hb))
                      for b in pair if b is not None][:len(hb) + len(tb)]

    with tile.TileContext(nc) as tc:
        with (
            tc.tile_pool(name="idx", bufs=4) as idx_pool,
            tc.tile_pool(name="gather", bufs=4) as g_pool,
            tc.tile_pool(name="dense", bufs=4) as d_pool,
            tc.tile_pool(name="trash", bufs=2) as trash_pool,
            tc.tile_pool(name="scores", bufs=1) as s_pool,
        ):
            scores = s_pool.tile([P, total_slots], f32)

            # --- positives: both sides dense ---
            slot = 0
            left = pos_slots
            col = 0
            while left > 0:
                n = min(left, BATCH_SLOTS)
                A = d_pool.tile([P, BATCH_SLOTS * D], f32, tag="pa")
                nc.sync.dma_start(A[:, 0:n * D],
                                  pos_a_d[:, col * D:(col + n) * D])
                B = d_pool.tile([P, BATCH_SLOTS * D], f32, tag="pb")
                nc.sync.dma_start(B[:, 0:n * D],
                                  pos_b_d[:, col * D:(col + n) * D])
                for s in range(n):
                    tr = trash_pool.tile([P, D], f32, tag="tr")
                    nc.vector.scalar_tensor_tensor(
                        out=tr[:], in0=A[:, s * D:(s + 1) * D], scalar=1.0,
                        in1=B[:, s * D:(s + 1) * D], op0=mult, op1=mult,
                        accum_out=scores[:, slot + s:slot + s + 1])
                col += n
                left -= n
                slot += n

            # --- head / tail: gather + dense ---
            # slot offsets: head section starts at pos_slots, tail after
            sec_base = {id(hidx_d): pos_slots, id(tidx_d): pos_slots + nh}
            for bi, (table, c, idx_d, dense_d, col, n) in enumerate(
                    gather_batches):
                q = bi % 4
                nidx = n * P
                cols = n * 8
                base = sec_base[id(idx_d)] + col
                ia = idx_pool.tile([P, BATCH_SLOTS * 8], i16, tag="ia")
                nc.sync.dma_start(ia[:, 0:cols],
                                  idx_d[:, col * 8:col * 8 + cols])
                G = g_pool.tile([P, BATCH_SLOTS * D], f32, tag="G")
                nc.gpsimd.dma_gather(
                    out_ap=G[:, 0:n * D].rearrange("p (g d) -> p g d", d=D),
                    in_ap=table[c * CHUNK:min((c + 1) * CHUNK, N_A), :],
                    idxs_ap=ia[:, 0:cols],
                    num_idxs=nidx, num_idxs_reg=nidx, elem_size=D,
                    queue_num=q)
                Dn = d_pool.tile([P, BATCH_SLOTS * D], f32, tag="dn")
                nc.sync.dma_start(Dn[:, 0:n * D],
                                  dense_d[:, col * D:(col + n) * D])
                for s in range(n):
                    tr = trash_pool.tile([P, D], f32, tag="tr")
                    nc.vector.scalar_tensor_tensor(
                        out=tr[:], in0=G[:, s * D:(s + 1) * D], scalar=1.0,
                        in1=Dn[:, s * D:(s + 1) * D], op0=mult, op1=mult,
                        accum_out=scores[:, base + s:base + s + 1])

            nc.sync.dma_start(s_out[:], scores[:])

    nc.compile()
    return nc


def _wrap_idx_batched(flat_idx, group_slots):
    """[S, P] int16 per-slot indices -> [P, S*8] dma_gather layout. Batch
    boundaries mirror the device program: per chunk-group, batches of up to
    BATCH_SLOTS slots; each batch's n*128 indices are 16-wrapped and
    replicated across the 8 Q7 cores."""
    S = flat_idx.shape[0]
    assert S == sum(group_slots)
    out = np.empty((P, S * 8), dtype=np.int16)
    col = 0
    s = 0
    for gs in group_slots:
        left = gs
        while left > 0:
            n = min(left, BATCH_SLOTS)
            flat = flat_idx[s:s + n].reshape(-1)       # slot-major, 128 fast
            w16 = flat.reshape(n * P // 16, 16).T      # [16, n*8]
            out[:, col:col + n * 8] = np.tile(w16, (8, 1))
            col += n * 8
            s += n
            left -= n
    return out


def _deal(padded_len, arrs):
    """Reshape [padded_len]-arrays to [slots, NCORES, P] dealt layout."""
    return [a.reshape(-1, NCORES, P) for a in arrs]


def kernel(emb_A, emb_B, rel_kernel, edge_pos, head_batch, tail_batch):
    from concourse.bass_utils import run_bass_kernel_spmd

    emb_A = np.ascontiguousarray(np.asarray(emb_A, dtype=np.float32))
    emb_B = np.ascontiguousarray(np.asarray(emb_B, dtype=np.float32))
    kv = np.asarray(rel_kernel, dtype=np.float32)[0]
    ep = np.asarray(edge_pos, dtype=np.int64)
    hb = np.asarray(head_batch, dtype=np.int64)
    tb = np.asarray(tail_batch, dtype=np.int64)

    # host-side prescaled row lookups (built lazily per needed rows)
    emb_Bk = emb_B * kv[None, :]
    emb_Ak = emb_A * kv[None, :]

    # ---------- positives ----------
    pos_pad = -(-E // SUB) * SUB
    pos_slots = pos_pad // SUB
    a_idx = np.zeros(pos_pad, np.int64)
    b_idx = np.zeros(pos_pad, np.int64)
    outp = np.full(pos_pad, -1, np.int64)
    a_idx[:E], b_idx[:E], outp[:E] = ep[0], ep[1], np.arange(E)
    a_s, b_s, o_s = _deal(pos_pad, [a_idx, b_idx, outp])

    # ---------- head / tail (sorted by corrupt-index chunk) ----------
    def section(corrupt_idx, shared_rows, out_base):
        """corrupt_idx [4E], shared_rows [4E,128] f32 (prescaled side),
        returns (group_slots, per-core idx arrays, dense arrays, outpos)."""
        npair = corrupt_idx.shape[0]
        key = corrupt_idx // CHUNK
        order = np.argsort(key, kind="stable")
        ci_s = corrupt_idx[order]
        op_s = out_base + order
        counts = np.bincount(key, minlength=NCHUNKS)
        group_slots = [int(-(-c // SUB)) for c in counts]
        idx_cores = [[] for _ in range(NCORES)]
        dense_cores = [[] for _ in range(NCORES)]
        outpos_cores = [[] for _ in range(NCORES)]
        start = 0
        for g in range(NCHUNKS):
            cnt = int(counts[g])
            padded = group_slots[g] * SUB
            gi = np.zeros(padded, np.int16)
            gp = np.full(padded, -1, np.int64)
            gi[:cnt] = (ci_s[start:start + cnt] - g * CHUNK).astype(np.int16)
            gp[:cnt] = op_s[start:start + cnt]
            gsh = np.zeros((padded,), np.int64)
            gsh[:cnt] = order[start:start + cnt]
            start += cnt
            gi3, gp3, gsh3 = _deal(padded, [gi, gp, gsh])
            for c in range(NCORES):
                idx_cores[c].append(gi3[:, c, :])
                outpos_cores[c].append(gp3[:, c, :].reshape(-1))
                dense_cores[c].append(gsh3[:, c, :])
        per_core = []
        for c in range(NCORES):
            idx_sp = np.concatenate(idx_cores[c], axis=0)        # [S, P]
            shared_sel = np.concatenate(dense_cores[c], axis=0)  # [S, P]
            dense = shared_rows[shared_sel]                      # [S, P, D]
            dense = np.ascontiguousarray(
                dense.transpose(1, 0, 2).reshape(P, -1))         # [P, S*D]
            per_core.append((
                np.ascontiguousarray(_wrap_idx_batched(idx_sp, group_slots)),
                dense,
                np.concatenate(outpos_cores[c]),
            ))
        return group_slots, per_core

    head_shared = emb_Bk[np.repeat(ep[1], NEG)]     # [4E, D]
    head_slots, head_pc = section(hb.reshape(-1), head_shared, E)
    tail_shared = emb_Ak[np.repeat(ep[0], NEG)]
    tail_slots, tail_pc = section(tb.reshape(-1), tail_shared, 5 * E)

    in_maps = []
    outpos_cores = []
    for c in range(NCORES):
        pos_a = np.ascontiguousarray(
            emb_A[a_s[:, c, :]].transpose(1, 0, 2).reshape(P, -1))
        pos_b = np.ascontiguousarray(
            emb_Bk[b_s[:, c, :]].transpose(1, 0, 2).reshape(P, -1))
        in_maps.append({
            "emb_a": emb_A,
            "emb_b": emb_B,
            "pos_a": pos_a,
            "pos_b": pos_b,
            "head_idx": head_pc[c][0],
            "head_dense": head_pc[c][1],
            "tail_idx": tail_pc[c][0],
            "tail_dense": tail_pc[c][1],
        })
        outpos_cores.append(np.concatenate(
            [o_s[:, c, :].reshape(-1), head_pc[c][2], tail_pc[c][2]]))

    sig = (pos_slots, tuple(head_slots), tuple(tail_slots))
    if _CACHED.get("sig") != sig:
        _CACHED["nc"] = _build_program(pos_slots, head_slots, tail_slots)
        _CACHED["sig"] = sig
    nc = _CACHED["nc"]
    _CACHED["in_maps"] = in_maps
    _CACHED["plan"] = sig

    res = run_bass_kernel_spmd(nc, in_maps, core_ids=list(range(NCORES)))
    _CACHED["last_results"] = res

    out = np.empty(9 * E, dtype=np.float32)
    for c in range(NCORES):
        flat = res.results[c]["scores"].T.reshape(-1)   # j = slot*128 + p
        op = outpos_cores[c]
        valid = op >= 0
        out[op[valid]] = flat[valid]
    return out


# revision 15
# speedup vs baseline: 5.7313x; 1.1398x over previous
"""DistMult edge-scoring kernel for Trainium2 (8 NeuronCores, SPMD).

score[j] = sum_d emb_A[a_idx[j], d] * k[d] * emb_B[b_idx[j], d]
for 9E pairs: E positive edges, 4E head-corrupted, 4E tail-corrupted.

Strategy (v3, hybrid dense/gather — exploits the repeat structure):
- The positive-edge rows and the repeated rows (b-side of head mode,
  a-side of tail mode, both k-prescaled on the host) are uploaded as
  DENSE per-pair arrays and streamed with plain HWDGE DMA.
- Only the corrupt heads/tails are gathered on-device via
  gpsimd.dma_gather (int16 chunk-local indices, tables split in 4
  chunks of 25000 rows, pairs sorted by chunk on the host). Gathers
  round-robin over 4 SWDGE queues (descriptor generation on the Q7
  cores is the bottleneck; 4 queues parallelize it).
- All 9E pairs are dealt round-robin across the 8 cores in 128-pair
  sub-slots so every core runs an identical instruction stream (true
  SPMD). The program is built after seeing the data; compile is cached
  on the group-slot signature.
- Compute: one fused scalar_tensor_tensor (mul + accumulate-reduce) per
  128-pair slot on the vector engine.
- Host inverse-permutes the scores back to reference order.
"""

import numpy as np

# problem constants
N_A = 100000
N_B = 100000
D = 128
E = 100000
NEG = 4
NCORES = 8

P = 128
CHUNK = 25000          # table rows per int16-indexable chunk
NCHUNKS = 4
BATCH_SLOTS = 8        # 128-pair slots per batch (num_idxs=1024 HW ceiling)
SUB = P * NCORES       # pairs per dealt slot-row (1024)

_CACHED = {}


def _build_program(pos_slots, head_slots, tail_slots):
    """head_slots/tail_slots: per-chunk slot counts (len 4). Same for all
    cores. Program: pos (dense+dense), head (gather-a + dense-b), tail
    (dense-a + gather-b)."""
    import concourse.tile as tile
    from concourse import bacc, mybir

    f32 = mybir.dt.float32
    i16 = mybir.dt.int16
    mult = mybir.AluOpType.mult

    nh = sum(head_slots)
    nt = sum(tail_slots)
    total_slots = pos_slots + nh + nt

    nc = bacc.Bacc("TRN2", target_bir_lowering=False, debug=False,
                   num_devices=NCORES, num_swdge_queues=4)
    embA = nc.dram_tensor("emb_a", [N_A, D], f32, kind="ExternalInput").ap()
    embB = nc.dram_tensor("emb_b", [N_B, D], f32, kind="ExternalInput").ap()
    pos_a_d = nc.dram_tensor("pos_a", [P, pos_slots * D], f32,
                             kind="ExternalInput").ap()
    pos_b_d = nc.dram_tensor("pos_b", [P, pos_slots * D], f32,
                             kind="ExternalInput").ap()
    hidx_d = nc.dram_tensor("head_idx", [P, nh * 8], i16,
                            kind="ExternalInput").ap()
    hdense_d = nc.dram_tensor("head_dense", [P, nh * D], f32,
                              kind="ExternalInput").ap()
    tidx_d = nc.dram_tensor("tail_idx", [P, nt * 8], i16,
                            kind="ExternalInput").ap()
    tdense_d = nc.dram_tensor("tail_dense", [P, nt * D], f32,
                              kind="ExternalInput").ap()
    s_out = nc.dram_tensor("scores", [P, total_slots], f32,
                           kind="ExternalOutput").ap()

    # (table_ap, chunk, idx dram, dense dram, idx col0, dense col0, n_slots)
    gather_batches = []

    def section_batches(slots_per_chunk, idx_d, dense_d, table):
        out = []
        col = 0
        for c, gs in enumerate(slots_per_chunk):
            left = gs
            while left > 0:
                n = min(left, BATCH_SLOTS)
                out.append((table, c, idx_d, dense_d, col, n))
                col += n
                left -= n
        return out

    hb = section_batches(head_slots, hidx_d, hdense_d, embA)
    tb = section_batches(tail_slots, tidx_d, tdense_d, embB)
    # interleave head/tail so both tables' gathers spread over queues
    gather_batches = [b for pair in
                      zip(hb + [None] * len(tb), tb + [None] * len(hb))
                      for b in pair if b is not None][:len(hb) + len(tb)]

    with tile.TileContext(nc) as tc:
        with (
            tc.tile_pool(name="idx", bufs=8) as idx_pool,
            tc.tile_pool(name="gather", bufs=8) as g_pool,
            tc.tile_pool(name="dense", bufs=6) as d_pool,
            tc.tile_pool(name="trash", bufs=2) as trash_pool,
            tc.tile_pool(name="scores", bufs=1) as s_pool,
        ):
            scores = s_pool.tile([P, total_slots], f32)

            # --- positives: both sides dense ---
            slot = 0
            left = pos_slots
            col = 0
            while left > 0:
                n = min(left, BATCH_SLOTS)
                A = d_pool.tile([P, BATCH_SLOTS * D], f32, tag="pa")
                nc.sync.dma_start(A[:, 0:n * D],
                                  pos_a_d[:, col * D:(col + n) * D])
                B = d_pool.tile([P, BATCH_SLOTS * D], f32, tag="pb")
                nc.sync.dma_start(B[:, 0:n * D],
                                  pos_b_d[:, col * D:(col + n) * D])
                for s in range(n):
                    tr = trash_pool.tile([P, D], f32, tag="tr")
                    nc.vector.scalar_tensor_tensor(
                        out=tr[:], in0=A[:, s * D:(s + 1) * D], scalar=1.0,
                        in1=B[:, s * D:(s + 1) * D], op0=mult, op1=mult,
                        accum_out=scores[:, slot + s:slot + s + 1])
                col += n
                left -= n
                slot += n

            # --- head / tail: gather + dense ---
            # slot offsets: head section starts at pos_slots, tail after
            sec_base = {id(hidx_d): pos_slots, id(tidx_d): pos_slots + nh}
            for bi, (table, c, idx_d, dense_d, col, n) in enumerate(
                    gather_batches):
                q = bi % 4
                nidx = n * P
                cols = n * 8
                base = sec_base[id(idx_d)] + col
                ia = idx_pool.tile([P, BATCH_SLOTS * 8], i16, tag="ia")
                nc.sync.dma_start(ia[:, 0:cols],
                                  idx_d[:, col * 8:col * 8 + cols])
                G = g_pool.tile([P, BATCH_SLOTS * D], f32, tag="G")
                nc.gpsimd.dma_gather(
                    out_ap=G[:, 0:n * D].rearrange("p (g d) -> p g d", d=D),
                    in_ap=table[c * CHUNK:min((c + 1) * CHUNK, N_A), :],
                    idxs_ap=ia[:, 0:cols],
                    num_idxs=nidx, num_idxs_reg=nidx, elem_size=D,
                    queue_num=q)
                Dn = d_pool.tile([P, BATCH_SLOTS * D], f32, tag="dn")
                nc.sync.dma_start(Dn[:, 0:n * D],
                                  dense_d[:, col * D:(col + n) * D])
                for s in range(n):
                    tr = trash_pool.tile([P, D], f32, tag="tr")
                    nc.vector.scalar_tensor_tensor(
                        out=tr[:], in0=G[:, s * D:(s + 1) * D], scalar=1.0,
                        in1=Dn[:, s * D:(s + 1) * D], op0=mult, op1=mult,
                        accum_out=scores[:, base + s:base + s + 1])

            nc.sync.dma_start(s_out[:], scores[:])

    nc.compile()
    return nc


def _wrap_idx_batched(flat_idx, group_slots):
    """[S, P] int16 per-slot indices -> [P, S*8] dma_gather layout. Batch
    boundaries mirror the device program: per chunk-group, batches of up to
    BATCH_SLOTS slots; each batch's n*128 indices are 16-wrapped and
    replicated across the 8 Q7 cores."""
    S = flat_idx.shape[0]
    assert S == sum(group_slots)
    out = np.empty((P, S * 8), dtype=np.int16)
    col = 0
    s = 0
    for gs in group_slots:
        left = gs
        while left > 0:
            n = min(left, BATCH_SLOTS)
            flat = flat_idx[s:s + n].reshape(-1)       # slot-major, 128 fast
            w16 = flat.reshape(n * P // 16, 16).T      # [16, n*8]
            out[:, col:col + n * 8] = np.tile(w16, (8, 1))
            col += n * 8
            s += n
            left -= n
    return out


def _deal(padded_len, arrs):
    """Reshape [padded_len]-arrays to [slots, NCORES, P] dealt layout."""
    return [a.reshape(-1, NCORES, P) for a in arrs]


def kernel(emb_A, emb_B, rel_kernel, edge_pos, head_batch, tail_batch):
    from concourse.bass_utils import run_bass_kernel_spmd

    emb_A = np.ascontiguousarray(np.asarray(emb_A, dtype=np.float32))
    emb_B = np.ascontiguousarray(np.asarray(emb_B, dtype=np.float32))
    kv = np.asarray(rel_kernel, dtype=np.float32)[0]
    ep = np.asarray(edge_pos, dtype=np.int64)
    hb = np.asarray(head_batch, dtype=np.int64)
    tb = np.asarray(tail_batch, dtype=np.int64)

    # host-side prescaled row lookups (built lazily per needed rows)
    emb_Bk = emb_B * kv[None, :]
    emb_Ak = emb_A * kv[None, :]

    # ---------- positives ----------
    pos_pad = -(-E // SUB) * SUB
    pos_slots = pos_pad // SUB
    a_idx = np.zeros(pos_pad, np.int64)
    b_idx = np.zeros(pos_pad, np.int64)
    outp = np.full(pos_pad, -1, np.int64)
    a_idx[:E], b_idx[:E], outp[:E] = ep[0], ep[1], np.arange(E)
    a_s, b_s, o_s = _deal(pos_pad, [a_idx, b_idx, outp])

    # ---------- head / tail (sorted by corrupt-index chunk) ----------
    def section(corrupt_idx, shared_rows, out_base):
        """corrupt_idx [4E], shared_rows [4E,128] f32 (prescaled side),
        returns (group_slots, per-core idx arrays, dense arrays, outpos)."""
        npair = corrupt_idx.shape[0]
        key = corrupt_idx // CHUNK
        order = np.argsort(key, kind="stable")
        ci_s = corrupt_idx[order]
        op_s = out_base + order
        counts = np.bincount(key, minlength=NCHUNKS)
        group_slots = [int(-(-c // SUB)) for c in counts]
        idx_cores = [[] for _ in range(NCORES)]
        dense_cores = [[] for _ in range(NCORES)]
        outpos_cores = [[] for _ in range(NCORES)]
        start = 0
        for g in range(NCHUNKS):
            cnt = int(counts[g])
            padded = group_slots[g] * SUB
            gi = np.zeros(padded, np.int16)
            gp = np.full(padded, -1, np.int64)
            gi[:cnt] = (ci_s[start:start + cnt] - g * CHUNK).astype(np.int16)
            gp[:cnt] = op_s[start:start + cnt]
            gsh = np.zeros((padded,), np.int64)
            gsh[:cnt] = order[start:start + cnt]
            start += cnt
            gi3, gp3, gsh3 = _deal(padded, [gi, gp, gsh])
            for c in range(NCORES):
                idx_cores[c].append(gi3[:, c, :])
                outpos_cores[c].append(gp3[:, c, :].reshape(-1))
                dense_cores[c].append(gsh3[:, c, :])
        per_core = []
        for c in range(NCORES):
            idx_sp = np.concatenate(idx_cores[c], axis=0)        # [S, P]
            shared_sel = np.concatenate(dense_cores[c], axis=0)  # [S, P]
            dense = shared_rows[shared_sel]                      # [S, P, D]
            dense = np.ascontiguousarray(
                dense.transpose(1, 0, 2).reshape(P, -1))         # [P, S*D]
            per_core.append((
                np.ascontiguousarray(_wrap_idx_batched(idx_sp, group_slots)),
                dense,
                np.concatenate(outpos_cores[c]),
            ))
        return group_slots, per_core

    head_shared = emb_Bk[np.repeat(ep[1], NEG)]     # [4E, D]
    head_slots, head_pc = section(hb.reshape(-1), head_shared, E)
    tail_shared = emb_Ak[np.repeat(ep[0], NEG)]
    tail_slots, tail_pc = section(tb.reshape(-1), tail_shared, 5 * E)

    in_maps = []
    outpos_cores = []
    for c in range(NCORES):
        pos_a = np.ascontiguousarray(
            emb_A[a_s[:, c, :]].transpose(1, 0, 2).reshape(P, -1))
        pos_b = np.ascontiguousarray(
            emb_Bk[b_s[:, c, :]].transpose(1, 0, 2).reshape(P, -1))
        in_maps.append({
            "emb_a": emb_A,
            "emb_b": emb_B,
            "pos_a": pos_a,
            "pos_b": pos_b,
            "head_idx": head_pc[c][0],
            "head_dense": head_pc[c][1],
            "tail_idx": tail_pc[c][0],
            "tail_dense": tail_pc[c][1],
        })
        outpos_cores.append(np.concatenate(
            [o_s[:, c, :].reshape(-1), head_pc[c][2], tail_pc[c][2]]))

    sig = (pos_slots, tuple(head_slots), tuple(tail_slots))
    if _CACHED.get("sig") != sig:
        _CACHED["nc"] = _build_program(pos_slots, head_slots, tail_slots)
        _CACHED["sig"] = sig
    nc = _CACHED["nc"]
    _CACHED["in_maps"] = in_maps
    _CACHED["plan"] = sig

    res = run_bass_kernel_spmd(nc, in_maps, core_ids=list(range(NCORES)))
    _CACHED["last_results"] = res

    out = np.empty(9 * E, dtype=np.float32)
    for c in range(NCORES):
        flat = res.results[c]["scores"].T.reshape(-1)   # j = slot*128 + p
        op = outpos_cores[c]
        valid = op >= 0
        out[op[valid]] = flat[valid]
    return out
